# revision 22
# baseline (speedup 1.0000x reference)
"""Trainium2 Bass kernel for nn_LinearTriParser (B=2,S=128,H=1024,A=256,C=14).

Math: score[b,i,j,k,c] = sh0[i,c]+st0[j,c]+sm0[k,c]; softmax over k in
[i,j]. sh0/st0/c0m drop out of the softmax, so with E=exp(sm0) and
eS=E*(sm1-mean):
  valid (i<=j): attn = mean + (sum_{k=i..j} eS[k,c]) / (sum_{k=i..j} E[k,c])
  invalid: attn = mean
final[b,i,j,c] = sh1[i,c]+st1[j,c]+uni[c] + attn.

Window sums are computed DIRECTLY (no prefix sums, no mid-kernel flats
DMAs): den[i,(j,c)] = sum_k step[k,i] * (tri[k,j]*E[k,c]) where
step[k,i]=[k>=i] is a constant fp8 lhsT and (tri*E) is a DVE
broadcast-multiply. Invalid windows: num is EXACTLY 0 (tri/step exact
0/1) and den gets a tiny positive floor via tri_den = tri + 1e-5*(1-tri),
so num/den = 0 there: no mask tensor, no max op.
st1[j,c] broadcast over i is delivered through the pB matmul: comb x lb
(sh1 + consts + mean, mean-trick) accumulated with ones x st1exp, where
st1exp[c',(j,c)] = st1[c',j]*eye[c',c] is one Pool broadcast-multiply --
no SBUF->SBUF flats DMA anywhere.

5 input DMAs (bm=m-branch, bt, bh fp8 on SP; tri2 bf16 consts, bcc f32r
on Act). All biases ride in tri2 (bf16). Engine queues are manually
scheduled with tile_wait_until floors (the auto-scheduler head-of-line
blocks pden behind h-L1 and rc before r2num otherwise). Tail:
rc=1/den (DVE, PSUM), at=num*rc, fin=at+pB in bf16 halves with split
output DMAs. Sharding: (batch b, j-quarter) per core; 8 SPMD programs.
"""

import numpy as np

B, S, H, A, C = 2, 128, 1024, 256, 14
P = 128
JW = 32            # j columns per core
W = JW * C         # 448 free width of cubic tiles
KH = H // P        # 8 k-tiles over the H contraction
DLT = 1e-5         # den floor on invalid windows

NM = 112 + 1024 + 2048        # bm: fh | mx k0-7 | w1m k0-7
NT = 128 + 256 + 2048         # bt: stepT | mxt | w1t
NH = 2048                     # bh: w1h
NCC = 128 + W + 128           # bcc f32r [32, 704]: lhsT_b | rhs_b | ones14
NTRI = 48                     # tri2 bf16: tri|ones|f32-packed consts
HW = W // 2                   # output half width


def _build():
    import concourse.mybir as mybir
    import concourse.tile as tile
    from concourse import bacc

    f32 = mybir.dt.float32
    f32r = mybir.dt.float32r
    bf16 = mybir.dt.bfloat16
    f8 = mybir.dt.float8e4
    AF = mybir.ActivationFunctionType
    OP = mybir.AluOpType

    nc = bacc.Bacc("TRN2", target_bir_lowering=False, debug=False,
                   enable_asserts=False, num_devices=8)

    bm = nc.dram_tensor("bm", [P, NM], f8, kind="ExternalInput")
    bt = nc.dram_tensor("bt", [P, NT], f8, kind="ExternalInput")
    bh = nc.dram_tensor("bh", [P, NH], f8, kind="ExternalInput")
    bcc = nc.dram_tensor("bcc", [32, NCC], f32r, kind="ExternalInput")
    tri2 = nc.dram_tensor("tri2", [P, NTRI], bf16, kind="ExternalInput")
    outp0 = nc.dram_tensor("outp0", [P, W], bf16, kind="ExternalOutput")

    MM = mybir.MatmulPerfMode.DoubleRow

    with tile.TileContext(nc) as tc:
        def at_(ns):
            return tc.tile_wait_until(ns / 1e6)

        with (
            tc.tile_pool(name="pers", bufs=1) as pers,
            tc.tile_pool(name="work", bufs=2) as work,
            tc.tile_pool(name="ps_a", bufs=1, space="PSUM") as ps_a,
            tc.tile_pool(name="ps_b", bufs=1, space="PSUM") as ps_b,
            tc.tile_pool(name="ps_hd", bufs=3, space="PSUM") as ps_hd,
            tc.tile_pool(name="ps_big", bufs=3, space="PSUM") as ps_big,
        ):
            ps_l1 = (ps_a, ps_b)
            # ---- t=0 prefetches: act table load + PE p-state anchor ----
            dum = pers.tile([1, 4], f32, name="dum", tag="dum")
            nc.vector.memset(dum[:], 1.0)
            nc.scalar.activation(dum[:, 0:1], dum[:, 0:1], AF.Exp, scale=1.0)
            pdum = ps_hd.tile([1, 1], f32, name="pdum", tag="hdps")
            for _ in range(3):
                nc.tensor.matmul(pdum[:], dum[:, 1:2], dum[:, 2:3],
                                 start=True, stop=True)
            ones128 = pers.tile([P, P], f8, name="ones", tag="ones")
            nc.gpsimd.memset(ones128[:], 1.0)
            epsr = pers.tile([1, W], bf16, name="epsr", tag="epsr")
            nc.gpsimd.memset(epsr[:], DLT)

            # ---- input DMAs (SP: big fp8; Act: consts) ----
            m_sb = pers.tile([P, NM], f8, name="msb", tag="msb")
            nc.sync.dma_start(m_sb[:], bm.ap())
            t_sb = pers.tile([P, NT], f8, name="tsb", tag="tsb")
            nc.sync.dma_start(t_sb[:], bt.ap())
            h_sb = pers.tile([P, NH], f8, name="hsb", tag="hsb")
            nc.sync.dma_start(h_sb[:], bh.ap())
            cc_sb = pers.tile([32, NCC], f32r, name="ccsb", tag="ccsb")
            nc.sync.dma_start(cc_sb[:], bcc.ap())
            tri_sb = pers.tile([P, NTRI], bf16, name="trisb", tag="trisb")
            nc.scalar.dma_start(tri_sb[:], tri2.ap())

            fh = m_sb[:, 0:112]
            # f32 consts bit-packed in bf16 cols 66:80:
            # cols (f32): 0=t_b1h0 1=t_b1h1 2=h0 3=h1 4=m0 5=m1 6=lbbias
            cst = tri_sb[:, 34:48].bitcast(f32)
            stepT = t_sb[:, 0:128]
            mxt = t_sb[:, 128:384]
            wt = t_sb[:, 384:NT]

            def mxp_t(t):
                c0 = 112 + t * 2 * P
                return m_sb[:, c0:c0 + 2 * P].rearrange(
                    "p (t n) -> p t n", t=2)

            def w1m_t(t, kk):
                c0 = 1136 + t * 512 + kk * 2 * P
                return m_sb[:, c0:c0 + 2 * P].rearrange(
                    "p (t m) -> p t m", t=2)

            # ---- m branch L1 (DoubleRow fp8) ----
            ym = pers.tile([P, 2 * P], f8, name="ym", tag="ym")
            psm = [ps_l1[kk].tile([P, P], f32, name=f"psm{kk}",
                                  tag=f"l1ps{kk}")
                   for kk in range(2)]
            for t in range(4):
                for kk in range(2):
                    nc.tensor.matmul(
                        psm[kk][:], w1m_t(t, kk), mxp_t(t),
                        start=(t == 0), stop=(t == 3), perf_mode=MM)
            with at_(4250):
                nc.vector.tensor_scalar(ym[:, 0:P], psm[0][:],
                                        cst[:, 4:5], 0.0, op0=OP.add,
                                        op1=OP.max)
                nc.scalar.activation(ym[:, P:2 * P], psm[1][:], AF.Relu,
                                     bias=cst[:, 5:6], scale=1.0)

            # ---- m heads in [k, c] layout via lhsT=ym ----
            ph0t = ps_hd.tile([P, C], f32, name="ph0t", tag="hdps")
            with at_(4800):
                for kk in range(2):
                    nc.tensor.matmul(
                        ph0t[:], ym[:, kk * P:(kk + 1) * P],
                        fh[:, 56 + kk * C: 56 + (kk + 1) * C],
                        start=(kk == 0), stop=(kk == 1))
            # e2 = [E_t | eS't] bf16 [128, 28]
            e2 = pers.tile([P, 2 * C], bf16, name="e2", tag="e2")
            with at_(4900):
                nc.scalar.activation(e2[:, 0:C], ph0t[:], AF.Exp,
                                     scale=1.0 / 1024.0)
            phm1t = ps_hd.tile([P, C], f32, name="phm1t", tag="hdps")
            with at_(4830):
                for kk in range(2):
                    nc.tensor.matmul(
                        phm1t[:], ym[:, kk * P:(kk + 1) * P],
                        fh[:, 84 + kk * C: 84 + (kk + 1) * C],
                        start=(kk == 0), stop=(kk == 1))
            sm1t = work.tile([P, C], bf16, name="sm1t", tag="sm1t")
            with at_(5000):
                nc.vector.tensor_scalar(sm1t[:], phm1t[:], 1.0 / 1024.0,
                                        0.0, op0=OP.mult, op1=OP.bypass)
            # meanB[i, c] = sum_k sm1t ; meanc[c, 0] = sum_k sm1t
            meanB = ps_hd.tile([P, C], f32, name="meanB", tag="hdps")
            meanc = ps_hd.tile([C, 1], f32, name="meanc", tag="hdps")
            with at_(5400):
                nc.tensor.matmul(meanB[:], ones128[:], sm1t[:],
                                 start=True, stop=True)
                nc.tensor.matmul(meanc[:], sm1t[:], tri_sb[:, 32:33],
                                 start=True, stop=True)
            # mcs2 = meanc/128 + (c1h+uni+c1m+c1t): full lb bias (Act)
            mcs2 = work.tile([C, 1], f32, name="mcs2", tag="mcs2")
            with at_(6650):
                nc.scalar.activation(mcs2[:], meanc[:], AF.Identity,
                                     bias=cst[0:C, 6:7], scale=1.0 / P)

            # ---- r2den = tri_den (x) E_t ; den matmul ----
            r2 = pers.tile([P, 2 * W], bf16, name="r2", tag="r2")
            with at_(5690), nc.allow_low_precision("bf16 vs 2e-2 tol"):
                nc.vector.tensor_tensor(
                    r2[:, 0:W].rearrange("p (q c) -> p q c", q=JW),
                    tri_sb[:, 0:JW].unsqueeze(2).broadcast_to([P, JW, C]),
                    e2[:, 0:C].unsqueeze(1).broadcast_to([P, JW, C]),
                    op=OP.mult)
            pden = ps_big.tile([P, W], f32, name="pden", tag="big")
            with at_(6050):
                nc.tensor.matmul(pden[:], stepT, r2[:, 0:W],
                                 start=True, stop=False)
                nc.tensor.matmul(pden[:], ones128[0:1, :], epsr[:],
                                 start=False, stop=True)

            # d1 = sm1t - meanB/128 ; r2num = r2den * d1 (q-broadcast)
            d1 = work.tile([P, C], bf16, name="d1", tag="d1")
            with at_(5400):
                nc.vector.scalar_tensor_tensor(
                    d1[:], meanB[:], -1.0 / P, sm1t[:],
                    op0=OP.mult, op1=OP.add)
            with at_(5900), nc.allow_low_precision("bf16 vs 2e-2 tol"):
                nc.vector.tensor_tensor(
                    r2[:, W:2 * W].rearrange("p (q c) -> p q c", q=JW),
                    r2[:, 0:W].rearrange("p (q c) -> p q c", q=JW),
                    d1[:].unsqueeze(1).broadcast_to([P, JW, C]),
                    op=OP.mult)
            pnum = ps_big.tile([P, W], f32, name="pnum", tag="big")
            with at_(6600):
                nc.tensor.matmul(pnum[:], stepT, r2[:, W:2 * W],
                                 start=True, stop=True)

            # ---- t branch L1 -> st1e [c, j] -> st1exp (Pool) ----
            yt = pers.tile([P, 2 * JW], f8, name="yt", tag="yt")
            pstl = []
            with at_(4990):
                for kk in range(2):
                    pst = ps_l1[kk].tile([P, JW], f32, name=f"pst{kk}",
                                         tag=f"l1ps{kk}")
                    pstl.append(pst)
                    for t in range(4):
                        nc.tensor.matmul(
                            pst[:],
                            wt[:, t * 512 + kk * 2 * P:
                               t * 512 + (kk + 1) * 2 * P]
                            .rearrange("p (t m) -> p t m", t=2),
                            mxt[:, t * 2 * JW:(t + 1) * 2 * JW]
                            .rearrange("p (t n) -> p t n", t=2),
                            start=(t == 0), stop=(t == 3), perf_mode=MM)
            with at_(5150):
                nc.vector.tensor_scalar(yt[:, 0:JW], pstl[0][:],
                                        cst[:, 0:1], 0.0, op0=OP.add,
                                        op1=OP.max)
            with at_(5150):
                nc.scalar.activation(yt[:, JW:2 * JW], pstl[1][:], AF.Relu,
                                     bias=cst[:, 1:2], scale=1.0)
            st1p = ps_hd.tile([C, JW], f32, name="st1p", tag="hdps")
            with at_(5800):
                for kk in range(2):
                    nc.tensor.matmul(
                        st1p[:], fh[:, 28 + kk * C: 28 + (kk + 1) * C],
                        yt[:, kk * JW:(kk + 1) * JW],
                        start=(kk == 0), stop=(kk == 1))
            st1e = work.tile([C, JW], f32r, name="st1e", tag="st1e")
            with at_(6600):
                nc.scalar.activation(st1e[:], st1p[:], AF.Copy,
                                     scale=1.0 / 1024.0)
            # st1exp[c', (q, c)] = st1e[c', q] * eye[c', c]  (Pool)
            st1x = pers.tile([C, W], f32r, name="st1x", tag="st1x")
            with at_(6700):
                nc.gpsimd.tensor_tensor(
                    st1x[:].rearrange("p (q c) -> p q c", q=JW),
                    st1e[:].unsqueeze(2).broadcast_to([C, JW, C]),
                    cc_sb[0:C, P:P + C].unsqueeze(1)
                    .broadcast_to([C, JW, C]),
                    op=OP.mult)

            # ---- h branch L1 -> lb into lhsT_b rows 0:13 ----
            yh = pers.tile([P, 2 * P], f8, name="yh", tag="yh")
            pshl = []
            with at_(5700):
                for kk in range(2):
                    psh = ps_l1[kk].tile([P, P], f32, name=f"psh{kk}",
                                         tag=f"l1ps{kk}")
                    pshl.append(psh)
                    for t in range(4):
                        nc.tensor.matmul(
                            psh[:],
                            h_sb[:, t * 512 + kk * 2 * P:
                                 t * 512 + (kk + 1) * 2 * P]
                            .rearrange("p (t m) -> p t m", t=2),
                            mxp_t(t),
                            start=(t == 0), stop=(t == 3), perf_mode=MM)
            with at_(6040):
                nc.scalar.activation(yh[:, 0:P], pshl[0][:], AF.Relu,
                                     bias=cst[:, 2:3], scale=1.0)
            with at_(6330):
                nc.scalar.activation(yh[:, P:2 * P], pshl[1][:], AF.Relu,
                                     bias=cst[:, 3:4], scale=1.0)
            phh = ps_hd.tile([C, P], f32, name="phh", tag="hdps")
            with at_(6450):
                nc.tensor.matmul(phh[:], fh[:, 0:C], yh[:, 0:P],
                                 start=True, stop=False)
            with at_(6750):
                nc.tensor.matmul(phh[:], fh[:, C:2 * C], yh[:, P:2 * P],
                                 start=False, stop=True)
            # lb = sh1/1024 + (consts + mean) -> lhsT_b rows 0:13
            with at_(6900):
                nc.scalar.activation(cc_sb[0:C, 0:P], phh[:], AF.Identity,
                                     bias=mcs2[:], scale=1.0 / 1024.0)

            # ---- pB = sh1B (comb x lb) + st1B (ones x st1exp) ----
            pB = ps_big.tile([P, W], f32, name="pB", tag="big")
            with at_(7150):
                nc.tensor.matmul(pB[:], cc_sb[0:C, 0:P],
                                 cc_sb[0:C, P:P + W],
                                 start=True, stop=False)
                nc.tensor.matmul(pB[:], cc_sb[0:C, P + W:2 * P + W], st1x[:],
                                 start=False, stop=True)

            # ---- tail: rc = 1/den, at = num*rc, fin = at + pB ----
            rc = work.tile([P, W], bf16, name="rc", tag="rc")
            with at_(6400), nc.allow_low_precision("bf16 vs 2e-2 tol"):
                nc.vector.reciprocal(rc[:], pden[:])
            at = work.tile([P, W], bf16, name="at", tag="at")
            with at_(7100):
                nc.vector.tensor_mul(at[:], rc[:], pnum[:])
            fin = work.tile([P, W], bf16, name="fin", tag="fin")
            with at_(7800):
                nc.vector.tensor_add(fin[:], at[:], pB[:])
            nc.sync.dma_start(outp0.ap(), fin[:])

    nc.finalize()
    return nc


_NC_CACHE = None


def _tile8(w):
    """[H, X] -> [128, 8*X]: col block k = rows 128k:128k+128."""
    return np.ascontiguousarray(
        w.reshape(KH, P, -1).transpose(1, 0, 2).reshape(P, -1))


def _tile8dr(w):
    """[H, A] -> [128, 2048] DoubleRow pack:
    cols = t*512 + kk*256 + d*128 + a' with (2t+d) the k-tile."""
    w8 = w.reshape(KH, P, 2, P)            # [k, p, kk, a']
    out = w8.reshape(4, 2, P, 2, P)        # [t, d, p, kk, a']
    out = out.transpose(2, 0, 3, 1, 4)     # [p, t, kk, d, a']
    return np.ascontiguousarray(out.reshape(P, 2048))


def kernel(**inputs):
    import ml_dtypes
    from concourse.bass_utils import run_bass_kernel_spmd

    global _NC_CACHE
    if _NC_CACHE is None:
        _NC_CACHE = _build()
    nc = _NC_CACHE

    bf16 = ml_dtypes.bfloat16
    f8 = ml_dtypes.float8_e4m3
    f32 = np.float32
    m = {k: np.asarray(v, f32) for k, v in inputs.items()}
    memory = m["memory"]

    # host-folded layer-2 + score heads
    F1h = m["h_W2"] @ m["s1h_W"] * 64.0
    c1h = m["h_b2"] @ m["s1h_W"] + m["s1h_b"] + m["uni"]
    F1t = m["t_W2"] @ m["s1t_W"] * 64.0
    c1t = m["t_b2"] @ m["s1t_W"] + m["s1t_b"]
    F0m = m["m_W2"] @ m["s0m_W"] * 64.0
    F1m = m["m_W2"] @ m["s1m_W"] * 64.0
    c1m = m["m_b2"] @ m["s1m_W"] + m["s1m_b"]

    fhp = np.concatenate(
        [F1h.reshape(2, P, C).transpose(1, 0, 2).reshape(P, 2 * C),
         F1t.reshape(2, P, C).transpose(1, 0, 2).reshape(P, 2 * C),
         F0m.reshape(2, P, C).transpose(1, 0, 2).reshape(P, 2 * C),
         F1m.reshape(2, P, C).transpose(1, 0, 2).reshape(P, 2 * C)],
        axis=1)  # [128, 112]

    w1m_p = _tile8dr(m["m_W1"] * 16.0)
    w1t_p = _tile8dr(m["t_W1"] * 16.0)
    w1h_p8 = np.asarray(_tile8dr(m["h_W1"] * 16.0), f8)

    stepT = (np.arange(P)[:, None] >= np.arange(P)[None, :]).astype(f32)

    # bcc [32, 704]: lhsT_b | rhs_b (comb rows 0:13) | ones14
    comb = (np.arange(C)[:, None, None] ==
            np.arange(C)[None, None, :]).astype(f32)  # [C,1,C]
    ccp = np.zeros((32, NCC), f32)
    ccp[0:C, P:P + W] = np.broadcast_to(comb, (C, JW, C)).reshape(C, W)
    ccp[0:C, P + W:NCC] = 1.0

    mxp = {}
    for b in range(B):
        mxp[b] = _tile8(np.ascontiguousarray(memory[b].T))  # [128, 1024]

    bmv = {}
    for b in range(B):
        bmv[b] = np.asarray(np.concatenate(
            [fhp, mxp[b], w1m_p], axis=1), f8)

    in_maps = []
    for cid in range(8):
        b, jq = cid // 4, cid % 4
        j0 = jq * JW

        tri = (np.arange(P)[:, None] <=
               (j0 + np.arange(JW))[None, :]).astype(f32)
        triv = np.zeros((P, NTRI), f32)
        triv[:, 0:JW] = tri
        triv[:, 32] = 1.0
        trib = triv.astype(bf16)
        cstv = np.zeros((P, 7), f32)
        cstv[:, 0] = m["t_b1"][0:P] * 16.0
        cstv[:, 1] = m["t_b1"][P:2 * P] * 16.0
        cstv[:, 2] = m["h_b1"][0:P] * 16.0
        cstv[:, 3] = m["h_b1"][P:2 * P] * 16.0
        cstv[:, 4] = m["m_b1"][0:P] * 16.0
        cstv[:, 5] = m["m_b1"][P:2 * P] * 16.0
        cstv[0:C, 6] = c1h + c1m + c1t  # uni already in c1h
        trib[:, 34:48] = cstv.view(np.uint16).view(bf16)

        mxt = memory[b, j0:j0 + JW].T.reshape(KH, P, JW)
        mxt = mxt.transpose(1, 0, 2).reshape(P, 8 * JW)
        btv = np.concatenate([stepT, mxt, w1t_p], axis=1)

        in_maps.append({
            "bm": bmv[b],
            "bt": np.asarray(btv, f8),
            "bh": w1h_p8,
            "bcc": ccp,
            "tri2": trib,
        })

    res = run_bass_kernel_spmd(nc, in_maps, core_ids=list(range(8)))
    out = np.zeros((B, S, S, C), dtype=f32)
    for cid in range(8):
        b, jq = cid // 4, cid % 4
        j0 = jq * JW
        out[b, :, j0:j0 + JW, :] = np.asarray(
            res.results[cid]["outp0"], f32).reshape(P, JW, C)
    return out
